# revision 46
# baseline (speedup 1.0000x reference)
"""FBPinn (16-window 1D PINN ensemble) forward pass on 8 Trainium2 NeuronCores.

Strategy (MoE-style routing, expert-parallel over windows):
  - Each of the 100k points lies strictly inside only 1-2 of the 16
    overlapping subdomains, so we route: gather the points of each window
    on the host (the routing is a pure function of x and the static
    geometry), run each window's small MLP only on its own points on
    device, and scatter-add the windowed outputs back on the host.
  - Core c owns windows (2c, 2c+1). The two windows are packed into the
    128-partition dimension (64 neurons each) so the hidden matmuls run
    with K=M=128 via block-diagonal weights and every tanh fills all 128
    ACT lanes.
  - Layer 1 is a K=2 outer-product matmul on per-window-normalized
    inputs xn = (x-mean)/std; biases are fused into the tanh ACT ops.
  - All matmul operands are float32r (1-pass reduced-precision fp32
    matmul, ~2x faster than true fp32 on the PE) with fp32 PSUM
    accumulation; tanh/bias math is full fp32 on the ACT engine.
  - The device pipeline is a skewed software pipeline (wavefront over
    point-chunks) so the ACT tanh stream runs gap-free while the PE,
    DVE (output combine) and DMA queues overlap underneath it.
  - The window routing weights win = sig((x-l)/s)*sig(-(x-r)/s) (0.03%
    of the model FLOPs) are computed host-side in float64 and fused into
    the device-side output combine (out + b/4) * 4*win on the DVE.
"""

import numpy as np

# Problem constants (mirrors reference.py static config)
NW = 16
D0, D1 = 0.0, 100.0
OVERLAP = 0.25
SIGMA = 0.5
NEURONS = 64
N = 100_000

NCORES = 8
NPAD = 8192          # per-window padded point count (max real count is 7930)
F = 1024             # chunk width (points) per tanh activation
NCHUNK = NPAD // F
NBLK = NPAD // 512   # 512-wide output blocks per window

_state: dict = {}


def _geometry():
    width = (D1 - D0) / NW
    i = np.arange(NW)
    lo = np.where(i == 0, D0, D0 + (i - OVERLAP / 2) * width)
    hi = np.where(i == NW - 1, D1, D0 + (i + 1 + OVERLAP / 2) * width)
    means = (lo + hi) / 2
    std = (hi - lo) / 2
    ovm = np.empty(NW + 1)
    ovm[0] = lo[0]
    ovm[NW] = hi[-1]
    ovm[1:NW] = (hi[:-1] + lo[1:]) / 2
    f32 = lambda a: np.asarray(a, np.float32)
    return f32(lo), f32(hi), f32(means), f32(std), f32(ovm)


def _build_nc():
    import concourse.bass as bass  # noqa: F401
    import concourse.tile as tile
    from concourse import bacc, mybir

    f32 = mybir.dt.float32
    f32r = mybir.dt.float32r  # 1-pass reduced-precision fp32 matmul
    AF = mybir.ActivationFunctionType
    ALU = mybir.AluOpType

    nc = bacc.Bacc("TRN2", target_bir_lowering=False, debug=False)

    # batched inputs: xin = xpair (normalized), wm = [lh0 | lh1 | lout],
    # cm = [x32 | wb], bv = [b_in | b_h0 | b_h1 | W_in]
    d_xin = nc.dram_tensor("xin", [2, NPAD], f32r, kind="ExternalInput")
    d_lin = nc.dram_tensor("lin", [2, 128], f32r, kind="ExternalInput")
    d_wm = nc.dram_tensor("wm", [128, 258], f32r, kind="ExternalInput")
    d_bv = nc.dram_tensor("bv", [128, 4], f32, kind="ExternalInput")
    d_win2 = nc.dram_tensor("win2", [2, NPAD], f32, kind="ExternalInput")
    d_bo2 = nc.dram_tensor("bo2", [2, 1], f32, kind="ExternalInput")
    d_out = nc.dram_tensor("out", [2, NPAD], f32, kind="ExternalOutput")

    with tile.TileContext(nc) as tc:
        with (
            tc.tile_pool(name="consts", bufs=1) as cp,
            tc.tile_pool(name="hp", bufs=4) as hp,
            tc.tile_pool(name="pp", bufs=3, space="PSUM") as pp,
            tc.tile_pool(name="pop", bufs=2, space="PSUM") as pop,
            tc.tile_pool(name="mp", bufs=1) as mp,
        ):
            xpr = cp.tile([2, NPAD + 128], f32r, tag="xpr")
            nc.sync.dma_start(xpr[:, NPAD:], d_lin[:])
            nc.sync.dma_start(xpr[:, 0:NPAD], d_xin[:])
            lin = xpr[:, NPAD:]
            wm = cp.tile([128, 258], f32r, tag="wm")
            nc.scalar.dma_start(wm[:], d_wm[:])
            bv = cp.tile([128, 4], f32, tag="bv")
            nc.sync.dma_start(bv[:], d_bv[:])
            bo2 = cp.tile([2, 1], f32, tag="bo2")
            nc.sync.dma_start(bo2[:], d_bo2[:])
            win2 = cp.tile([2, NPAD], f32, tag="win2")
            nc.sync.dma_start(win2[:], d_win2[:])

            lh0 = wm[:, 0:128]
            lh1 = wm[:, 128:256]
            lout = wm[:, 256:258]

            fin2 = mp.tile([2, NPAD], f32, tag="fin2")
            h3big = mp.tile([128, NPAD], f32r, tag="h3big")


            def emit_out(j):
                for s in range(F // 512):
                    n = j * (F // 512) + s
                    sl = slice(n * 512, (n + 1) * 512)
                    pout = pop.tile([2, 512], f32, tag="po", name=f"po_{n}")
                    nc.tensor.matmul(
                        pout[:], lout, h3big[:, sl], start=True, stop=True,
                    )
                    # fused combine: fin = (out + b_out/4) * 4*win
                    nc.vector.scalar_tensor_tensor(
                        fin2[:, sl], pout[:], bo2[:, 0:1], win2[:, sl],
                        op0=ALU.add, op1=ALU.mult,
                    )
                    nc.sync.dma_start(d_out[:, sl], fin2[:, sl])

            # ---- skewed software pipeline (wavefront over chunks) ----
            h1s, h2s = {}, {}

            def stage1(j):
                xsl = slice(j * F, (j + 1) * F)
                p1 = pp.tile([128, F], f32, tag="ps", name=f"p1_{j}")
                for s in range(F // 512):
                    sl = slice(s * 512, (s + 1) * 512)
                    xs = slice(j * F + s * 512, j * F + (s + 1) * 512)
                    nc.tensor.matmul(p1[:, sl], lin, xpr[:, xs], start=True, stop=True)
                h1 = hp.tile([128, F], f32r, tag="h1", name=f"h1_{j}")
                nc.scalar.activation(h1[:], p1[:], AF.Tanh, bias=bv[:, 0:1])
                h1s[j] = h1

            def stage2(j):
                p2 = pp.tile([128, F], f32, tag="ps", name=f"p2_{j}")
                for s in range(F // 512):
                    sl = slice(s * 512, (s + 1) * 512)
                    nc.tensor.matmul(p2[:, sl], lh0, h1s[j][:, sl], start=True, stop=True)
                h2 = hp.tile([128, F], f32r, tag="h2", name=f"h2_{j}")
                nc.scalar.activation(h2[:], p2[:], AF.Tanh, bias=bv[:, 1:2])
                h2s[j] = h2

            def stage3(j):
                p3 = pp.tile([128, F], f32, tag="ps", name=f"p3_{j}")
                for s in range(F // 512):
                    sl = slice(s * 512, (s + 1) * 512)
                    nc.tensor.matmul(p3[:, sl], lh1, h2s[j][:, sl], start=True, stop=True)
                nc.scalar.activation(
                    h3big[:, j * F : (j + 1) * F], p3[:], AF.Tanh, bias=bv[:, 2:3]
                )
                emit_out(j)

            for t in range(NCHUNK + 2):
                if t < NCHUNK:
                    stage1(t)
                if 1 <= t < NCHUNK + 1:
                    stage2(t - 1)
                if t >= 2:
                    stage3(t - 2)


    nc.compile()
    return nc


def _get_nc():
    if "nc" not in _state:
        _state["nc"] = _build_nc()
    return _state["nc"]


def _prepare(x, W_in, b_in, W_h, b_h, W_out, b_out):
    x = np.asarray(x, np.float32)
    W_in = np.asarray(W_in, np.float32)
    b_in = np.asarray(b_in, np.float32)
    W_h = np.asarray(W_h, np.float32)
    b_h = np.asarray(b_h, np.float32)
    W_out = np.asarray(W_out, np.float32)
    b_out = np.asarray(b_out, np.float32)

    lo, hi, means, std, ovm = _geometry()

    # ---- host routing: gather each window's points ----
    idxs, counts = [], []
    for w in range(NW):
        idx = np.nonzero((lo[w] < x) & (x < hi[w]))[0]
        assert len(idx) <= NPAD, f"window {w} has {len(idx)} points > NPAD={NPAD}"
        idxs.append(idx)
        counts.append(len(idx))

    in_maps = []
    for c in range(NCORES):
        A, B = 2 * c, 2 * c + 1
        xA = np.full(NPAD, means[A], np.float32)
        xA[: counts[A]] = x[idxs[A]]
        xB = np.full(NPAD, means[B], np.float32)
        xB[: counts[B]] = x[idxs[B]]
        # normalized per-window inputs (matches reference's xn exactly)
        xin = np.stack([(xA - means[A]) / std[A], (xB - means[B]) / std[B]])

        lin = np.zeros((2, 128), np.float32)
        lin[0, :64] = W_in[A]
        lin[1, 64:] = W_in[B]

        bv = np.empty((128, 4), np.float32)
        bv[:64, 0] = b_in[A]
        bv[64:, 0] = b_in[B]
        bv[:64, 1] = b_h[0, A]
        bv[64:, 1] = b_h[0, B]
        bv[:64, 2] = b_h[1, A]
        bv[64:, 2] = b_h[1, B]
        bv[:64, 3] = W_in[A]
        bv[64:, 3] = W_in[B]

        # wm = [lh0 | lh1 | lout]
        wm = np.zeros((128, 258), np.float32)
        wm[:64, 0:64] = W_h[0, A]
        wm[64:, 64:128] = W_h[0, B]
        wm[:64, 128:192] = W_h[1, A]
        wm[64:, 192:256] = W_h[1, B]
        wm[:64, 256] = W_out[A] * 0.25
        wm[64:, 257] = W_out[B] * 0.25

        # window routing weights, host-side (float64 sigmoids), scaled by 4
        # to match the b_out/4, W_out/4 folding: fin = (out + b/4) * (4*win)
        def win4_of(xw, w):
            z1 = 1.0 / (1.0 + np.exp(-(xw.astype(np.float64) - ovm[w]) / SIGMA))
            z2 = 1.0 / (1.0 + np.exp((xw.astype(np.float64) - ovm[w + 1]) / SIGMA))
            return (4.0 * z1 * z2).astype(np.float32)

        win2 = np.stack([win4_of(xA, A), win4_of(xB, B)])

        bo2 = np.array([[b_out[A] * 0.25], [b_out[B] * 0.25]], np.float32)

        in_maps.append(
            {"xin": xin, "lin": lin, "wm": wm, "bv": bv, "win2": win2, "bo2": bo2}
        )

    return in_maps, idxs, counts


def _postprocess(results, idxs, counts):
    pred = np.zeros(N, np.float32)
    for w in range(NW):
        c, s = divmod(w, 2)
        vals = results[c]["out"][s, : counts[w]]
        pred[idxs[w]] += vals
    return pred


def kernel(x, W_in, b_in, W_h, b_h, W_out, b_out):
    from concourse.bass_utils import run_bass_kernel_spmd

    in_maps, idxs, counts = _prepare(x, W_in, b_in, W_h, b_h, W_out, b_out)
    nc = _get_nc()
    res = run_bass_kernel_spmd(nc, in_maps, core_ids=list(range(NCORES)))
    return _postprocess(res.results, idxs, counts)


# revision 47
# speedup vs baseline: 1.0387x; 1.0387x over previous
"""FBPinn (16-window 1D PINN ensemble) forward pass on 8 Trainium2 NeuronCores.

Strategy (MoE-style routing, expert-parallel over windows):
  - Each of the 100k points lies strictly inside only 1-2 of the 16
    overlapping subdomains, so we route: gather the points of each window
    on the host (the routing is a pure function of x and the static
    geometry), run each window's small MLP only on its own points on
    device, and scatter-add the windowed outputs back on the host.
  - Core c owns windows (2c, 2c+1). The two windows are packed into the
    128-partition dimension (64 neurons each) so the hidden matmuls run
    with K=M=128 via block-diagonal weights and every tanh fills all 128
    ACT lanes.
  - Layer 1 is a K=2 outer-product matmul on per-window-normalized
    inputs xn = (x-mean)/std; biases are fused into the tanh ACT ops.
  - All matmul operands are float32r (1-pass reduced-precision fp32
    matmul, ~2x faster than true fp32 on the PE) with fp32 PSUM
    accumulation; tanh/bias math is full fp32 on the ACT engine.
  - The device pipeline is a skewed software pipeline (wavefront over
    point-chunks) so the ACT tanh stream runs gap-free while the PE,
    DVE (output combine) and DMA queues overlap underneath it.
  - The window routing weights win = sig((x-l)/s)*sig(-(x-r)/s) (0.03%
    of the model FLOPs) are computed host-side in float64 and fused into
    the device-side output combine (out + b/4) * 4*win on the DVE.
"""

import numpy as np

# Problem constants (mirrors reference.py static config)
NW = 16
D0, D1 = 0.0, 100.0
OVERLAP = 0.25
SIGMA = 0.5
NEURONS = 64
N = 100_000

NCORES = 8
NPAD = 8192          # per-window padded point count (max real count is 7930)
F = 1024             # chunk width (points) per tanh activation
NCHUNK = NPAD // F
NBLK = NPAD // 512   # 512-wide output blocks per window

_state: dict = {}


def _geometry():
    width = (D1 - D0) / NW
    i = np.arange(NW)
    lo = np.where(i == 0, D0, D0 + (i - OVERLAP / 2) * width)
    hi = np.where(i == NW - 1, D1, D0 + (i + 1 + OVERLAP / 2) * width)
    means = (lo + hi) / 2
    std = (hi - lo) / 2
    ovm = np.empty(NW + 1)
    ovm[0] = lo[0]
    ovm[NW] = hi[-1]
    ovm[1:NW] = (hi[:-1] + lo[1:]) / 2
    f32 = lambda a: np.asarray(a, np.float32)
    return f32(lo), f32(hi), f32(means), f32(std), f32(ovm)


def _build_nc():
    import concourse.bass as bass  # noqa: F401
    import concourse.tile as tile
    from concourse import bacc, mybir

    f32 = mybir.dt.float32
    f32r = mybir.dt.float32r  # 1-pass reduced-precision fp32 matmul
    AF = mybir.ActivationFunctionType
    ALU = mybir.AluOpType

    nc = bacc.Bacc("TRN2", target_bir_lowering=False, debug=False)

    # batched inputs: xin = xpair (normalized), wm = [lh0 | lh1 | lout],
    # cm = [x32 | wb], bv = [b_in | b_h0 | b_h1 | W_in]
    d_xin = nc.dram_tensor("xin", [2, NPAD], f32r, kind="ExternalInput")
    d_lin = nc.dram_tensor("lin", [2, 128], f32r, kind="ExternalInput")
    d_wm = nc.dram_tensor("wm", [128, 258], f32r, kind="ExternalInput")
    d_bv = nc.dram_tensor("bv", [128, 4], f32, kind="ExternalInput")
    d_win2 = nc.dram_tensor("win2", [2, NPAD], f32, kind="ExternalInput")
    d_bo2 = nc.dram_tensor("bo2", [2, 1], f32, kind="ExternalInput")
    d_out = nc.dram_tensor("out", [2, NPAD], f32, kind="ExternalOutput")

    with tile.TileContext(nc) as tc:
        with (
            tc.tile_pool(name="consts", bufs=1) as cp,
            tc.tile_pool(name="hp", bufs=4) as hp,
            tc.tile_pool(name="pp", bufs=3, space="PSUM") as pp,
            tc.tile_pool(name="pop", bufs=2, space="PSUM") as pop,
            tc.tile_pool(name="mp", bufs=1) as mp,
        ):
            xpr = cp.tile([2, NPAD + 128], f32r, tag="xpr")
            nc.sync.dma_start(xpr[:, NPAD:], d_lin[:])
            nc.sync.dma_start(xpr[:, 0:NPAD], d_xin[:])
            lin = xpr[:, NPAD:]
            wm = cp.tile([128, 258], f32r, tag="wm")
            nc.scalar.dma_start(wm[:], d_wm[:])
            bv = cp.tile([128, 4], f32, tag="bv")
            nc.sync.dma_start(bv[:], d_bv[:])
            bo2 = cp.tile([2, 1], f32, tag="bo2")
            nc.sync.dma_start(bo2[:], d_bo2[:])
            win2 = cp.tile([2, NPAD], f32, tag="win2")
            nc.sync.dma_start(win2[:], d_win2[:])

            lh0 = wm[:, 0:128]
            lh1 = wm[:, 128:256]
            lout = wm[:, 256:258]

            fin2 = mp.tile([2, NPAD], f32, tag="fin2")
            h3big = mp.tile([128, NPAD], f32r, tag="h3big")


            def emit_out(j):
                for s in range(F // 512):
                    n = j * (F // 512) + s
                    sl = slice(n * 512, (n + 1) * 512)
                    pout = pop.tile([2, 512], f32, tag="po", name=f"po_{n}")
                    nc.tensor.matmul(
                        pout[:], lout, h3big[:, sl], start=True, stop=True,
                    )
                    # fused combine: fin = (out + b_out/4) * 4*win
                    nc.vector.scalar_tensor_tensor(
                        fin2[:, sl], pout[:], bo2[:, 0:1], win2[:, sl],
                        op0=ALU.add, op1=ALU.mult,
                    )
                    nc.sync.dma_start(d_out[:, sl], fin2[:, sl])

            # ---- skewed software pipeline (wavefront over chunks) ----
            h1s, h2s = {}, {}

            def stage1(j):
                xsl = slice(j * F, (j + 1) * F)
                p1 = pp.tile([128, F], f32, tag="ps", name=f"p1_{j}")
                for s in range(F // 512):
                    sl = slice(s * 512, (s + 1) * 512)
                    xs = slice(j * F + s * 512, j * F + (s + 1) * 512)
                    nc.tensor.matmul(p1[:, sl], lin, xpr[:, xs], start=True, stop=True)
                h1 = hp.tile([128, F], f32r, tag="h1", name=f"h1_{j}")
                nc.scalar.activation(h1[:], p1[:], AF.Tanh, bias=bv[:, 0:1])
                h1s[j] = h1

            def stage2(j):
                p2 = pp.tile([128, F], f32, tag="ps", name=f"p2_{j}")
                for s in range(F // 512):
                    sl = slice(s * 512, (s + 1) * 512)
                    nc.tensor.matmul(p2[:, sl], lh0, h1s[j][:, sl], start=True, stop=True)
                h2 = hp.tile([128, F], f32r, tag="h2", name=f"h2_{j}")
                nc.scalar.activation(h2[:], p2[:], AF.Tanh, bias=bv[:, 1:2])
                h2s[j] = h2

            def stage3(j):
                p3 = pp.tile([128, F], f32, tag="ps", name=f"p3_{j}")
                for s in range(F // 512):
                    sl = slice(s * 512, (s + 1) * 512)
                    nc.tensor.matmul(p3[:, sl], lh1, h2s[j][:, sl], start=True, stop=True)
                nc.scalar.activation(
                    h3big[:, j * F : (j + 1) * F], p3[:], AF.Tanh, bias=bv[:, 2:3]
                )
                # out-stage for the PREVIOUS chunk: its tanh3 completed last
                # wavefront, so the in-order PE queue never stalls on ACT here
                if j >= 1:
                    emit_out(j - 1)

            for t in range(NCHUNK + 2):
                if t < NCHUNK:
                    stage1(t)
                if 1 <= t < NCHUNK + 1:
                    stage2(t - 1)
                if t >= 2:
                    stage3(t - 2)
            emit_out(NCHUNK - 1)


    nc.compile()
    return nc


def _get_nc():
    if "nc" not in _state:
        _state["nc"] = _build_nc()
    return _state["nc"]


def _prepare(x, W_in, b_in, W_h, b_h, W_out, b_out):
    x = np.asarray(x, np.float32)
    W_in = np.asarray(W_in, np.float32)
    b_in = np.asarray(b_in, np.float32)
    W_h = np.asarray(W_h, np.float32)
    b_h = np.asarray(b_h, np.float32)
    W_out = np.asarray(W_out, np.float32)
    b_out = np.asarray(b_out, np.float32)

    lo, hi, means, std, ovm = _geometry()

    # ---- host routing: gather each window's points ----
    idxs, counts = [], []
    for w in range(NW):
        idx = np.nonzero((lo[w] < x) & (x < hi[w]))[0]
        assert len(idx) <= NPAD, f"window {w} has {len(idx)} points > NPAD={NPAD}"
        idxs.append(idx)
        counts.append(len(idx))

    in_maps = []
    for c in range(NCORES):
        A, B = 2 * c, 2 * c + 1
        xA = np.full(NPAD, means[A], np.float32)
        xA[: counts[A]] = x[idxs[A]]
        xB = np.full(NPAD, means[B], np.float32)
        xB[: counts[B]] = x[idxs[B]]
        # normalized per-window inputs (matches reference's xn exactly)
        xin = np.stack([(xA - means[A]) / std[A], (xB - means[B]) / std[B]])

        lin = np.zeros((2, 128), np.float32)
        lin[0, :64] = W_in[A]
        lin[1, 64:] = W_in[B]

        bv = np.empty((128, 4), np.float32)
        bv[:64, 0] = b_in[A]
        bv[64:, 0] = b_in[B]
        bv[:64, 1] = b_h[0, A]
        bv[64:, 1] = b_h[0, B]
        bv[:64, 2] = b_h[1, A]
        bv[64:, 2] = b_h[1, B]
        bv[:64, 3] = W_in[A]
        bv[64:, 3] = W_in[B]

        # wm = [lh0 | lh1 | lout]
        wm = np.zeros((128, 258), np.float32)
        wm[:64, 0:64] = W_h[0, A]
        wm[64:, 64:128] = W_h[0, B]
        wm[:64, 128:192] = W_h[1, A]
        wm[64:, 192:256] = W_h[1, B]
        wm[:64, 256] = W_out[A] * 0.25
        wm[64:, 257] = W_out[B] * 0.25

        # window routing weights, host-side (float64 sigmoids), scaled by 4
        # to match the b_out/4, W_out/4 folding: fin = (out + b/4) * (4*win)
        def win4_of(xw, w):
            z1 = 1.0 / (1.0 + np.exp(-(xw.astype(np.float64) - ovm[w]) / SIGMA))
            z2 = 1.0 / (1.0 + np.exp((xw.astype(np.float64) - ovm[w + 1]) / SIGMA))
            return (4.0 * z1 * z2).astype(np.float32)

        win2 = np.stack([win4_of(xA, A), win4_of(xB, B)])

        bo2 = np.array([[b_out[A] * 0.25], [b_out[B] * 0.25]], np.float32)

        in_maps.append(
            {"xin": xin, "lin": lin, "wm": wm, "bv": bv, "win2": win2, "bo2": bo2}
        )

    return in_maps, idxs, counts


def _postprocess(results, idxs, counts):
    pred = np.zeros(N, np.float32)
    for w in range(NW):
        c, s = divmod(w, 2)
        vals = results[c]["out"][s, : counts[w]]
        pred[idxs[w]] += vals
    return pred


def kernel(x, W_in, b_in, W_h, b_h, W_out, b_out):
    from concourse.bass_utils import run_bass_kernel_spmd

    in_maps, idxs, counts = _prepare(x, W_in, b_in, W_h, b_h, W_out, b_out)
    nc = _get_nc()
    res = run_bass_kernel_spmd(nc, in_maps, core_ids=list(range(NCORES)))
    return _postprocess(res.results, idxs, counts)
